# revision 1
# baseline (speedup 1.0000x reference)
"""Trainium2 Bass kernel for nn_AuxiliaryClustering (segment_reduce).

Data-parallel over the batch dim on 8 NeuronCores: each core streams its
125k-row shard of latent_z / cluster_assignments once, computing
  - per-cluster counts + per-cluster sums of ||z_i - c_{argmax_i}||  (PE
    matmuls with a one-hot lhsT; gather of the selected center is done by
    DMA-transposing the one-hot and matmul'ing against block-diag C)
  - per-column sums of cluster_assignments (GpSimd elementwise
    accumulate, one PE reduction at the end)
  - the K x K center-separation row sums (tiny, computed redundantly)
The [K]-sized partials are gathered to the host, summed across cores
(the "all-reduce"), and the five scalar outputs are assembled there.
"""

import os
from contextlib import ExitStack

import ml_dtypes
import numpy as np

import concourse.bass as bass
import concourse.bacc as bacc
import concourse.tile as tile
from concourse import mybir
from concourse.bass_utils import run_bass_kernel_spmd

F32 = mybir.dt.float32
BF16 = mybir.dt.bfloat16
AX = mybir.AxisListType
OP = mybir.AluOpType
ACTF = mybir.ActivationFunctionType

B, D, K = 1000000, 64, 64
NCORES = 8
P = 128           # partitions
R = 16            # rows per partition per big tile
TILE_ROWS = P * R                                   # 2048
SHARD = B // NCORES                                 # 125000
NTILES = (SHARD + TILE_ROWS - 1) // TILE_ROWS       # 62
PAD_SHARD = NTILES * TILE_ROWS                      # 126976
NPAD = PAD_SHARD - SHARD                            # 1976
NCHUNK = R * D // P                                 # 8 chunks (r-pairs) per tile

EPS = 1e-08
WEIGHT = 0.1


def build_nc(ntiles: int = NTILES, pad_shard: int = PAD_SHARD):
    nc = bacc.Bacc("TRN2", target_bir_lowering=False, debug=False)

    a_d = nc.dram_tensor("a", [pad_shard, K], F32, kind="ExternalInput").ap()
    z_d = nc.dram_tensor("z", [pad_shard, D], BF16, kind="ExternalInput").ap()
    # block-diagonal [[C,0],[0,C]] in bf16 for the r-pair-merged gather matmul
    cbf_d = nc.dram_tensor("cbf", [P, P], BF16, kind="ExternalInput").ap()
    c_d = nc.dram_tensor("c", [K, D], F32, kind="ExternalInput").ap()
    ct_d = nc.dram_tensor("ct", [D, K], F32, kind="ExternalInput").ap()
    mask_d = nc.dram_tensor("mask", [K, K], F32, kind="ExternalInput").ap()
    ident_d = nc.dram_tensor("ident", [P, P], BF16, kind="ExternalInput").ap()

    # out: cols 0:4 = partials [dist0,cnt0,dist1,cnt1]; col4 colsum; col5 sep
    outk_d = nc.dram_tensor("out_k", [P, 8], F32, kind="ExternalOutput").ap()

    a_4d = a_d.rearrange("(t p r) d -> t p r d", p=P, r=R)
    z_4d = z_d.rearrange("(t p r) d -> t p r d", p=P, r=R)

    with tile.TileContext(nc) as tc, ExitStack() as ctx:
        iop = ctx.enter_context(tc.tile_pool(name="io", bufs=4))
        wp = ctx.enter_context(tc.tile_pool(name="work", bufs=4))
        cp = ctx.enter_context(tc.tile_pool(name="const", bufs=1))
        ps_acc = ctx.enter_context(tc.tile_pool(name="ps_acc", bufs=1, space="PSUM"))
        ps_oh = ctx.enter_context(tc.tile_pool(name="ps_oh", bufs=2, space="PSUM"))
        ps_cs = ctx.enter_context(tc.tile_pool(name="ps_cs", bufs=2, space="PSUM"))

        # --- constants ---
        cbf_t = cp.tile([P, P], BF16)
        nc.sync.dma_start(out=cbf_t[:], in_=cbf_d[:])
        ident_t = cp.tile([P, P], BF16)
        nc.sync.dma_start(out=ident_t[:], in_=ident_d[:])
        ones_t = cp.tile([P, 1], F32)
        nc.vector.memset(ones_t[:], 1.0)
        acc_t = cp.tile([P, R, K], F32)       # running per-(p,r,k) colsum acc
        nc.gpsimd.memset(acc_t[:], 0.0)

        # =====================  separation loss (tiny)  =====================
        sep_t = cp.tile([K, 1], F32)
        c_t = cp.tile([K, D], F32)
        nc.sync.dma_start(out=c_t[:], in_=c_d[:])
        ct_t = cp.tile([D, K], F32)
        nc.sync.dma_start(out=ct_t[:], in_=ct_d[:])
        mask_t = cp.tile([K, K], F32)
        nc.sync.dma_start(out=mask_t[:], in_=mask_d[:])

        csq_t = cp.tile([K, D], F32)
        nc.vector.tensor_tensor(out=csq_t[:], in0=c_t[:], in1=c_t[:], op=OP.mult)
        csqc_t = cp.tile([K, 1], F32)
        nc.vector.reduce_sum(csqc_t[:], csq_t[:], axis=AX.X)
        ctsq_t = cp.tile([D, K], F32)
        nc.vector.tensor_tensor(out=ctsq_t[:], in0=ct_t[:], in1=ct_t[:], op=OP.mult)

        g_ps = ps_acc.tile([K, K], F32, tag="sep_ps")
        nc.tensor.matmul(g_ps[:], ct_t[:], ct_t[:], start=True, stop=True)
        row_ps = ps_acc.tile([1, K], F32, tag="sep_row")
        nc.tensor.matmul(row_ps[:], ones_t[0:D, :], ctsq_t[:], start=True, stop=True)

        t1_t = cp.tile([K, K], F32)
        nc.scalar.activation(
            out=t1_t[:], in_=g_ps[:], func=ACTF.Identity,
            bias=csqc_t[:], scale=-2.0,
        )
        csqr_sb = cp.tile([1, K], F32)
        nc.scalar.copy(out=csqr_sb[:], in_=row_ps[:])
        csqr_b = cp.tile([K, K], F32)
        nc.gpsimd.partition_broadcast(csqr_b[:], csqr_sb[:])
        d2m_t = cp.tile([K, K], F32)
        nc.vector.tensor_tensor(out=d2m_t[:], in0=t1_t[:], in1=csqr_b[:], op=OP.add)
        nc.vector.tensor_scalar_max(out=d2m_t[:], in0=d2m_t[:], scalar1=0.0)
        dm_t = cp.tile([K, K], F32)
        nc.scalar.sqrt(dm_t[:], d2m_t[:])
        nc.vector.tensor_tensor(out=dm_t[:], in0=dm_t[:], in1=mask_t[:], op=OP.mult)
        nc.vector.reduce_sum(sep_t[:], dm_t[:], axis=AX.X)

        # --- accumulators (PSUM, reusing the one-shot separation slots) ---
        papp_ps = ps_acc.tile([P, 4], F32, tag="sep_row")   # merged partials

        # =====================  main loop over big tiles  ====================
        for i in range(ntiles):
            a_t = iop.tile([P, R, K], F32, tag="a")
            nc.sync.dma_start(out=a_t[:], in_=a_4d[i])
            # row max over K (per (p, r) row) -- DVE
            m_t = wp.tile([P, R, 1], F32, tag="m")
            nc.vector.reduce_max(m_t[:], a_t[:], axis=AX.X)

            # one-hot argmax (bf16) -- DVE
            oh_t = wp.tile([P, R, K], BF16, tag="oh")
            nc.vector.tensor_tensor(
                out=oh_t[:], in0=a_t[:],
                in1=m_t[:].broadcast_to([P, R, K]),
                op=OP.is_equal,
            )

            # transpose one-hot chunks on the PE (bf16 -> PSUM), copy via ACT
            oh2d = oh_t[:].rearrange("p r d -> p (r d)")
            ohT_ps = ps_oh.tile([P, NCHUNK, P], BF16, tag="ohTp")
            for j in range(NCHUNK):
                nc.tensor.transpose(
                    out=ohT_ps[:, j, :],
                    in_=oh2d[:, j * P:(j + 1) * P],
                    identity=ident_t[:],
                )
            ohT_t = wp.tile([P, NCHUNK, P], BF16, tag="ohT")
            nc.scalar.copy(out=ohT_t[:], in_=ohT_ps[:])

            # gather selected centers: one matmul per r-pair (N=128)
            csel_ps = ps_cs.tile([P, NCHUNK, P], F32, tag="csel")
            for j in range(NCHUNK):
                nc.tensor.matmul(
                    csel_ps[:, j, :],
                    ohT_t[:, j, :],
                    cbf_t[:],
                    start=True, stop=True,
                )
            # downcast copy PSUM->SBUF (weights already negated: holds -c_sel)
            diff_t = wp.tile([P, R, D], BF16, tag="diff")
            nc.scalar.copy(out=diff_t[:].rearrange("p r d -> p (r d)"),
                           in_=csel_ps[:].rearrange("p c q -> p (c q)"))
            # diff = z + (-c_sel): software-DGE DMA with accumulate
            nc.gpsimd.dma_start(out=diff_t[:], in_=z_4d[i], accum_op=OP.add)
            sq_t = wp.tile([P, R, D], BF16, tag="sq")
            nc.scalar.square(sq_t[:], diff_t[:])
            d2_t = wp.tile([P, R, 1], BF16, tag="d2")
            with nc.allow_low_precision("dist2 in bf16 is fine for this loss"):
                nc.vector.reduce_sum(d2_t[:], sq_t[:], axis=AX.X)

            do_t = wp.tile([P, R, 2], BF16, tag="do")
            nc.vector.memset(do_t[:, :, 1:2], 1.0)
            nc.scalar.sqrt(do_t[:, :, 0:1], d2_t[:])

            # merged per-cluster partials: one matmul per r-pair (N=4)
            do2d = do_t[:].rearrange("p r c -> p (r c)")
            for j in range(NCHUNK):
                nc.tensor.matmul(
                    papp_ps[:],
                    oh2d[:, j * P:(j + 1) * P],
                    do2d[:, j * 4:(j + 1) * 4],
                    start=(i == 0 and j == 0),
                    stop=(i == ntiles - 1 and j == NCHUNK - 1),
                )

            # colsum accumulate -- GpSimd (last: keep z-dma off the critical path)
            nc.gpsimd.tensor_tensor(
                out=acc_t[:], in0=a_t[:], in1=acc_t[:], op=OP.add,
            )

        # =====================  final colsum reduction  =====================
        colred_t = cp.tile([P, K], F32)
        acc_rk = acc_t[:].rearrange("p r d -> p (r d)").rearrange(
            "p (r d) -> p d r", d=K)
        nc.vector.reduce_sum(colred_t[:], acc_rk, axis=AX.X)
        cs_ps = ps_acc.tile([K, 1], F32, tag="sep_ps")
        nc.tensor.matmul(cs_ps[:], colred_t[:], ones_t[:], start=True, stop=True)

        # =====================  write outputs  =====================
        resk_t = cp.tile([P, 8], F32)
        nc.vector.memset(resk_t[:], 0.0)
        nc.vector.tensor_copy(out=resk_t[:, 0:4], in_=papp_ps[:])
        nc.vector.tensor_copy(out=resk_t[0:K, 4:5], in_=cs_ps[:])
        nc.vector.tensor_copy(out=resk_t[0:K, 5:6], in_=sep_t[:])
        nc.sync.dma_start(out=outk_d[:], in_=resk_t[:])

    nc.finalize()
    return nc


_NC_CACHE = {}


def _get_nc():
    if "nc" not in _NC_CACHE:
        _NC_CACHE["nc"] = build_nc()
    return _NC_CACHE["nc"]


def make_inputs(a_s, z_s, c):
    """Per-core input map from padded fp32 A shard, bf16 Z shard, centers."""
    cbf1 = c.astype(ml_dtypes.bfloat16)
    cbf = np.zeros((P, P), dtype=ml_dtypes.bfloat16)
    cbf[:K, :D] = -cbf1
    cbf[K:, D:] = -cbf1
    return {
        "a": a_s, "z": z_s, "cbf": cbf, "c": c,
        "ct": np.ascontiguousarray(c.T),
        "mask": (1.0 - np.eye(K, dtype=np.float32)),
        "ident": np.eye(P, dtype=np.float32).astype(ml_dtypes.bfloat16),
    }


def kernel(latent_z, cluster_assignments, cluster_centers):
    z = np.asarray(latent_z, dtype=np.float32)
    a = np.ascontiguousarray(np.asarray(cluster_assignments, dtype=np.float32))
    c = np.ascontiguousarray(np.asarray(cluster_centers, dtype=np.float32))

    cbf1 = c.astype(ml_dtypes.bfloat16)
    zb = z.astype(ml_dtypes.bfloat16)

    # pad rows: A = e0 (argmax 0, single max), Z = bf16(C[0]) so dist == 0
    a_pad_row = np.zeros((K,), dtype=np.float32)
    a_pad_row[0] = 1.0
    z_pad_row = cbf1[0]

    in_maps = []
    for core in range(NCORES):
        lo, hi = core * SHARD, (core + 1) * SHARD
        a_s = np.empty((PAD_SHARD, K), dtype=np.float32)
        z_s = np.empty((PAD_SHARD, D), dtype=ml_dtypes.bfloat16)
        a_s[:SHARD] = a[lo:hi]
        z_s[:SHARD] = zb[lo:hi]
        a_s[SHARD:] = a_pad_row
        z_s[SHARD:] = z_pad_row
        in_maps.append(make_inputs(a_s, z_s, c))

    nc = _get_nc()
    trace = bool(int(os.environ.get("KERNEL_PROFILE", "0")))
    res = run_bass_kernel_spmd(
        nc, in_maps, list(range(NCORES)), trace=trace, trace_cores=[0],
    )
    if trace:
        _NC_CACHE["exec_time_ns"] = res.exec_time_ns
        print(f"HW exec time: {res.exec_time_ns} ns")

    # ---- host-side all-reduce of partials + final scalar math ----
    outk = np.stack([r["out_k"] for r in res.results])        # [8, P, 8]

    dist_sum = (outk[:, :K, 0] + outk[:, K:, 2]).sum(axis=0).astype(np.float64)
    counts = (outk[:, :K, 1] + outk[:, K:, 3]).sum(axis=0).astype(np.float64)
    colsum = outk[:, :K, 4].sum(axis=0).astype(np.float64)
    sep_rowsum = outk[0, :K, 5].astype(np.float64)

    # remove padding contributions (pad rows all land in cluster 0)
    counts[0] -= NCORES * NPAD
    colsum[0] -= NCORES * NPAD

    probs = colsum / B
    balance = float(np.sum((1.0 / K) * (np.log(1.0 / K) - np.log(probs + EPS))))
    separation = float(-np.sum(sep_rowsum) / (K * (K - 1)))
    nonempty = counts > 0
    per_mean = dist_sum / np.maximum(counts, 1.0)
    n_nonempty = float(nonempty.sum())
    compact = float(np.sum(np.where(nonempty, per_mean, 0.0)) / max(n_nonempty, 1.0))
    aux = WEIGHT * balance + WEIGHT * separation + WEIGHT * compact
    cluster_balance = float(np.std(probs, ddof=1))

    return (
        np.float32(aux),
        np.float32(balance),
        np.float32(separation),
        np.float32(compact),
        np.float32(cluster_balance),
    )

